# revision 3
# baseline (speedup 1.0000x reference)
"""Trainium2 Bass kernel for sparse multi-head edge attention.

Computation (per the nn.Module):
    Q = Fa @ Wq.T, K = Fb @ Wk.T, V = Fb @ Wv.T   (reshaped to H=8 heads x 32)
    per edge e: logit[e,h] = <Q[a_e,h,:], K[b_e,h,:]> / sqrt(32)
    segmented softmax over edges per query, out = Fa + (softmax-weighted V) @ Wproj.T

Strategy (8 NeuronCores, SPMD, no collectives):
  - Shard queries: core m owns rows [m*6250, (m+1)*6250). Every core builds the
    full fused [K|V] table (fp16, 1KB rows) in its DRAM; the segmented softmax
    is fully core-local. exp() without max-subtraction (|logit| <~ 8).
  - Edge slots are grouped per query block (49 blocks of 128 queries), split
    into a LO region (keys < 32768) and HI region (keys >= 32768) so gather
    indices fit int16. Slot layout: [blk0-lo | blk1-lo | ... | blk0-hi | ...].
    Gathers are issued in ~20 multi-block chunks (~5k rows each) to amortize
    the ~2us/call overhead; descriptor generation on the GPSIMD Q7 is the
    kernel's critical resource (~8.5ns/row).
  - Per 128-edge tile: sel[p,q] = (a_rel[slot p] == q) one-hot built on DVE;
    selT = PE-transpose(sel); Qe = selT.T @ Qblk on the PE; logits via fp16
    halving-tree (2x DVE mode) instead of 1x tensor_reduce; den|num
    accumulated as one fused [ex | ex*V] matmul into PSUM per tile.
    All DVE/scalar ops are batched over NT=4 tiles to amortize op overheads.
  - Pad slots point at table row 0 with a_rel = 200: the one-hot column is
    all-zero so they contribute nothing (no bias machinery needed).
"""

import math

import numpy as np

P = 128
H = 8
DH = 32
CDIM = 256
NA = 50000
NB = 50000
NCORES = 8
NAC = NA // NCORES          # 6250 queries per core
NBLK = (NAC + P - 1) // P   # 49 query blocks per core
NPADQ = NBLK * P            # 6272 padded queries per core
SPLIT = 32768               # int16-safe table split
KV_ROWS = ((NB + P - 1) // P) * P   # 50048
KVHI_ROWS = KV_ROWS - SPLIT         # 17280
CHUNK = 2048                # rows per table-build chunk
SCALE = 1.0 / math.sqrt(DH)
PAD_AREL = 200.0            # out-of-range query id for pad slots
NT = 4                      # tiles per op slab
GCH = 40                    # max tiles per gather chunk

F16 = np.float16
F32 = np.float32


def _ceil128(x):
    return (np.asarray(x) + P - 1) // P * P


def preprocess(Fa, Fb, a_idx, b_idx, Wq, Wk, Wv, Wproj):
    """Host-side sharding: returns (meta, shared_inputs, per_core_inputs)."""
    a_idx = np.asarray(a_idx).astype(np.int64)
    b_idx = np.asarray(b_idx).astype(np.int64)
    Fa = np.asarray(Fa, F32)
    Fb = np.asarray(Fb, F32)

    core = a_idx // NAC
    a_loc = a_idx - core * NAC
    blk = a_loc // P
    a_rel_v = a_loc % P
    hi = b_idx >= SPLIT

    # per (core, block) lo/hi counts -> shared static capacities
    cnt_lo = np.zeros((NCORES, NBLK), np.int64)
    cnt_hi = np.zeros((NCORES, NBLK), np.int64)
    np.add.at(cnt_lo, (core[~hi], blk[~hi]), 1)
    np.add.at(cnt_hi, (core[hi], blk[hi]), 1)
    LO = _ceil128(cnt_lo.max(axis=0))
    HI = _ceil128(cnt_hi.max(axis=0))
    loff = np.concatenate([[0], np.cumsum(LO)])
    hoff = np.concatenate([[0], np.cumsum(HI)])
    TOTLO = int(loff[-1])
    TOTHI = int(hoff[-1])
    TOT = TOTLO + TOTHI
    TC = TOT // P

    # rank of each edge within its (core, blk, half) group
    ne = a_idx.shape[0]
    gid = (core * NBLK + blk) * 2 + hi.astype(np.int64)
    order = np.argsort(gid, kind="stable")
    counts = np.bincount(gid, minlength=NCORES * NBLK * 2)
    gstart = np.concatenate([[0], np.cumsum(counts)])[:-1]
    rank = np.empty(ne, np.int64)
    rank[order] = np.arange(ne) - gstart[gid[order]]

    # slot within the core's edge stream: lo region then hi region
    slot = np.where(hi, TOTLO + hoff[blk] + rank, loff[blk] + rank)

    kvlo_idx = np.zeros((NCORES, TOTLO), np.int16)
    kvhi_idx = np.zeros((NCORES, TOTHI), np.int16)
    arel = np.full((NCORES, TOT), PAD_AREL, F16)

    arel[core, slot] = a_rel_v.astype(F16)
    lo_m = ~hi
    kvlo_idx[core[lo_m], slot[lo_m]] = b_idx[lo_m].astype(np.int16)
    kvhi_idx[core[hi], slot[hi] - TOTLO] = (b_idx[hi] - SPLIT).astype(np.int16)

    def wrap16(arr):  # [N] -> [128, N/16] (16-slot wrap replicated 8x)
        w = arr.reshape(-1, 16).T
        return np.tile(w, (8, 1)).copy()

    def slots128(arr):  # [TOT] -> [128, TC]; slot i -> (i%128, i//128)
        return arr.reshape(-1, P).T.copy()

    FbT = np.zeros((CDIM, KV_ROWS), F16)
    FbT[:, :NB] = Fb.T.astype(F16)

    shared = {
        "FbT": FbT,
        "WqT": Wq.T.astype(F16).copy(),
        "WKVT": np.concatenate([Wk.T, Wv.T], axis=1).astype(F16).copy(),
        "WprojT": Wproj.T.astype(F16).copy(),
        "IOTA": np.tile(np.arange(P, dtype=F16), (P, 1)).copy(),
        "IDENT": np.eye(P, dtype=F16),
    }

    per_core = []
    for m in range(NCORES):
        FaT = np.zeros((CDIM, NPADQ), F16)
        FaT[:, :NAC] = Fa[m * NAC:(m + 1) * NAC].T.astype(F16)
        Fa_res = np.zeros((NPADQ, CDIM), F32)
        Fa_res[:NAC] = Fa[m * NAC:(m + 1) * NAC]
        per_core.append({
            "FaT": FaT,
            "FaRes": Fa_res,
            "KVLOIDX": wrap16(kvlo_idx[m]) if TOTLO else np.zeros((P, 1), np.int16),
            "KVHIIDX": wrap16(kvhi_idx[m]) if TOTHI else np.zeros((P, 1), np.int16),
            "AREL": slots128(arel[m]),
        })

    meta = {
        "LO": LO.astype(int), "HI": HI.astype(int),
        "loff": loff.astype(int), "hoff": hoff.astype(int),
        "TOT": TOT, "TC": TC, "TOTLO": TOTLO, "TOTHI": TOTHI,
    }
    return meta, shared, per_core


def _chunks_of_blocks(seg_tiles):
    """Group consecutive blocks (block -> segment tile count) into gather
    chunks of at most GCH tiles. Returns list of (j0, nblocks, ntiles)."""
    out = []
    j0 = 0
    cur = 0
    for j, t in enumerate(seg_tiles):
        if cur and cur + t > GCH:
            out.append((j0, j - j0, cur))
            j0, cur = j, 0
        cur += t
    if cur:
        out.append((j0, len(seg_tiles) - j0, cur))
    return out


def build_program(meta):
    import concourse.bacc as bacc
    import concourse.mybir as mybir
    from concourse.tile import TileContext
    from concourse import library_config

    dt = mybir.dt
    nc = bacc.Bacc("TRN2", target_bir_lowering=False, debug=False,
                   num_devices=NCORES)

    TOTLO, TOTHI = meta["TOTLO"], meta["TOTHI"]
    LO, HI = meta["LO"], meta["HI"]
    loff, hoff = meta["loff"], meta["hoff"]
    TC = meta["TC"]

    # ---- I/O ----
    FbT_t = nc.dram_tensor("FbT", [CDIM, KV_ROWS], dt.float16, kind="ExternalInput")
    FaT_t = nc.dram_tensor("FaT", [CDIM, NPADQ], dt.float16, kind="ExternalInput")
    FaRes_t = nc.dram_tensor("FaRes", [NPADQ, CDIM], dt.float32, kind="ExternalInput")
    WqT_t = nc.dram_tensor("WqT", [CDIM, CDIM], dt.float16, kind="ExternalInput")
    WKVT_t = nc.dram_tensor("WKVT", [CDIM, 2 * CDIM], dt.float16, kind="ExternalInput")
    WprojT_t = nc.dram_tensor("WprojT", [CDIM, CDIM], dt.float16, kind="ExternalInput")
    IOTA_t = nc.dram_tensor("IOTA", [P, P], dt.float16, kind="ExternalInput")
    IDENT_t = nc.dram_tensor("IDENT", [P, P], dt.float16, kind="ExternalInput")
    KVLO_I_t = nc.dram_tensor("KVLOIDX", [P, max(TOTLO // 16, 1)], dt.int16,
                              kind="ExternalInput")
    KVHI_I_t = nc.dram_tensor("KVHIIDX", [P, max(TOTHI // 16, 1)], dt.int16,
                              kind="ExternalInput")
    AREL_t = nc.dram_tensor("AREL", [P, TC], dt.float16, kind="ExternalInput")

    KVlo = nc.dram_tensor("KVlo", [SPLIT, 2 * CDIM], dt.float16, kind="Internal")
    KVhi = nc.dram_tensor("KVhi", [KVHI_ROWS, 2 * CDIM], dt.float16, kind="Internal")
    OUT_t = nc.dram_tensor("OUT", [NPADQ, CDIM], dt.float32, kind="ExternalOutput")

    AluOp = mybir.AluOpType
    Act = mybir.ActivationFunctionType

    lo_chunks = _chunks_of_blocks([int(LO[j]) // P for j in range(NBLK)])
    hi_chunks = _chunks_of_blocks([int(HI[j]) // P for j in range(NBLK)])

    with TileContext(nc) as tc:
        nc.gpsimd.load_library(library_config.mlp)
        with tc.tile_pool(name="res", bufs=1) as rpool:
            wq = rpool.tile([P, 2, CDIM], dt.float16, tag="wq")
            wkv = rpool.tile([P, 2, 2 * CDIM], dt.float16, tag="wkv")
            wproj = rpool.tile([P, 2, CDIM], dt.float16, tag="wproj")
            nc.sync.dma_start(out=wq[:, 0, :], in_=WqT_t[0:P, :])
            nc.sync.dma_start(out=wq[:, 1, :], in_=WqT_t[P:2 * P, :])
            nc.sync.dma_start(out=wkv[:, 0, :], in_=WKVT_t[0:P, :])
            nc.sync.dma_start(out=wkv[:, 1, :], in_=WKVT_t[P:2 * P, :])
            nc.sync.dma_start(out=wproj[:, 0, :], in_=WprojT_t[0:P, :])
            nc.sync.dma_start(out=wproj[:, 1, :], in_=WprojT_t[P:2 * P, :])
            iota = rpool.tile([P, P], dt.float16, tag="iota")
            ident = rpool.tile([P, P], dt.float16, tag="ident")
            nc.sync.dma_start(out=iota[:], in_=IOTA_t[:, :])
            nc.sync.dma_start(out=ident[:], in_=IDENT_t[:, :])
            kvloidx = rpool.tile([P, max(TOTLO // 16, 1)], dt.int16, tag="kvloidx")
            nc.sync.dma_start(out=kvloidx[:], in_=KVLO_I_t[:, :])
            kvhiidx = rpool.tile([P, max(TOTHI // 16, 1)], dt.int16, tag="kvhiidx")
            nc.sync.dma_start(out=kvhiidx[:], in_=KVHI_I_t[:, :])
            arel = rpool.tile([P, TC], dt.float16, tag="arel")
            nc.sync.dma_start(out=arel[:], in_=AREL_t[:, :])
            qres = rpool.tile([P, NBLK, CDIM], dt.float16, tag="qres")
            # den|num accumulators per block, SBUF-resident f32
            acc = rpool.tile([P, NBLK, H + CDIM], dt.float32, tag="acc")

            # ---- Phase A1: lo half of the fused KV table ----
            with tc.tile_pool(name="bld", bufs=2) as bpool, \
                 tc.tile_pool(name="psA", bufs=4, space="PSUM") as psA:
                def build_kv(row0, row1, dst, dst0):
                    for c0 in range(row0, row1, CHUNK):
                        nsub = min(CHUNK, row1 - c0) // P
                        ft = bpool.tile([P, 2, CHUNK], dt.float16, tag="ft")
                        nc.sync.dma_start(out=ft[:, 0, :nsub * P],
                                          in_=FbT_t[0:P, c0:c0 + nsub * P])
                        nc.sync.dma_start(out=ft[:, 1, :nsub * P],
                                          in_=FbT_t[P:2 * P, c0:c0 + nsub * P])
                        ob = bpool.tile([P, CHUNK // P, 2 * CDIM], dt.float16,
                                        tag="ob")
                        for s in range(nsub):
                            ps = psA.tile([P, 2 * CDIM], dt.float32, tag="psA")
                            nc.tensor.matmul(ps[:], ft[:, 0, s * P:(s + 1) * P],
                                             wkv[:, 0, :], start=True, stop=False)
                            nc.tensor.matmul(ps[:], ft[:, 1, s * P:(s + 1) * P],
                                             wkv[:, 1, :], start=False, stop=True)
                            nc.scalar.copy(out=ob[:, s, :], in_=ps[:])
                        nc.sync.dma_start(
                            out=dst[c0 - dst0:c0 - dst0 + nsub * P, :]
                                .rearrange("(s p) d -> p s d", p=P),
                            in_=ob[:, :nsub, :])

                build_kv(0, SPLIT, KVlo, 0)
                # ---- Phase A2 (emitted next; overlaps lo gathers): hi table, Q
                build_kv(SPLIT, KV_ROWS, KVhi, SPLIT)
                for c0 in range(0, NPADQ, CHUNK):
                    nsub = min(CHUNK, NPADQ - c0) // P
                    ft = bpool.tile([P, 2, CHUNK], dt.float16, tag="ft")
                    nc.sync.dma_start(out=ft[:, 0, :nsub * P],
                                      in_=FaT_t[0:P, c0:c0 + nsub * P])
                    nc.sync.dma_start(out=ft[:, 1, :nsub * P],
                                      in_=FaT_t[P:2 * P, c0:c0 + nsub * P])
                    for s in range(nsub):
                        ps = psA.tile([P, CDIM], dt.float32, tag="psAq")
                        nc.tensor.matmul(ps[:], ft[:, 0, s * P:(s + 1) * P],
                                         wq[:, 0, :], start=True, stop=False)
                        nc.tensor.matmul(ps[:], ft[:, 1, s * P:(s + 1) * P],
                                         wq[:, 1, :], start=False, stop=True)
                        nc.scalar.copy(out=qres[:, c0 // P + s, :], in_=ps[:])

            # ---- Phase B: edge attention ----
            with tc.tile_pool(name="gat", bufs=2) as gpool, \
                 tc.tile_pool(name="wrk", bufs=2) as wpool, \
                 tc.tile_pool(name="fin", bufs=2) as fpool, \
                 tc.tile_pool(name="psQ", bufs=1, space="PSUM") as psQ, \
                 tc.tile_pool(name="psT", bufs=2, space="PSUM") as psT, \
                 tc.tile_pool(name="psD", bufs=1, space="PSUM") as psD, \
                 tc.tile_pool(name="psF", bufs=1, space="PSUM") as psF:

                def do_slab(kve, arel_g0, kt0, j, nt):
                    """Process nt (<=NT) tiles of block j.
                    kve tile cols [kt0, kt0+nt); arel cols [arel_g0, ...)."""
                    sel = wpool.tile([P, NT, P], dt.float16, tag="sel")
                    nc.vector.tensor_tensor(
                        out=sel[:, 0:nt, :],
                        in0=arel[:, arel_g0:arel_g0 + nt].unsqueeze(2)
                            .to_broadcast([P, nt, P]),
                        in1=iota[:].unsqueeze(1).to_broadcast([P, nt, P]),
                        op=AluOp.is_equal)
                    selT_ps = psT.tile([P, NT, P], dt.float16, tag="selT_ps")
                    for t in range(nt):
                        nc.tensor.transpose(selT_ps[:, t, :], sel[:, t, :],
                                            ident[:])
                    selT = wpool.tile([P, NT, P], dt.float16, tag="selT")
                    nc.scalar.copy(out=selT[:, 0:nt, :], in_=selT_ps[:, 0:nt, :])
                    qe_ps = psQ.tile([P, NT, CDIM], dt.float32, tag="qe_ps")
                    for t in range(nt):
                        nc.tensor.matmul(qe_ps[:, t, :], selT[:, t, :],
                                         qres[:, j, :], start=True, stop=True)
                    qe = wpool.tile([P, NT, CDIM], dt.float16, tag="qe")
                    nc.scalar.copy(out=qe[:, 0:nt, :], in_=qe_ps[:, 0:nt, :])
                    prod = wpool.tile([P, NT, CDIM], dt.float16, tag="prod")
                    nc.vector.tensor_tensor(
                        out=prod[:, 0:nt, :], in0=qe[:, 0:nt, :],
                        in1=kve[:, kt0:kt0 + nt, 0:CDIM], op=AluOp.mult)
                    # halving tree over d: 32 -> 16 -> 8 -> 4 -> 2 -> 1
                    lt = wpool.tile([P, NT, H, 16], dt.float16, tag="lt")
                    pv = prod[:, 0:nt, :].rearrange("p t (h d) -> p t h d", d=DH)
                    nc.vector.tensor_tensor(
                        out=lt[:, 0:nt, :, :], in0=pv[:, :, :, 0:16],
                        in1=pv[:, :, :, 16:32], op=AluOp.add)
                    w = 16
                    while w > 1:
                        nc.vector.tensor_tensor(
                            out=lt[:, 0:nt, :, 0:w // 2],
                            in0=lt[:, 0:nt, :, 0:w // 2],
                            in1=lt[:, 0:nt, :, w // 2:w], op=AluOp.add)
                        w //= 2
                    exwv = wpool.tile([P, NT, H + CDIM], dt.float16, tag="exwv")
                    nc.scalar.activation(
                        out=exwv[:, 0:nt, 0:H],
                        in_=lt[:, 0:nt, :, 0], func=Act.Exp, scale=SCALE)
                    nc.vector.tensor_tensor(
                        out=exwv[:, 0:nt, H:H + CDIM],
                        in0=kve[:, kt0:kt0 + nt, CDIM:2 * CDIM],
                        in1=exwv[:, 0:nt, 0:H].unsqueeze(3)
                            .to_broadcast([P, nt, H, DH]),
                        op=AluOp.mult)
                    return sel, exwv

                def do_region(region_chunks, idx_tile, table, roff_tiles, is_lo):
                    """roff_tiles: block -> first tile index of its segment
                    within the region; region tile index -> arel col offset."""
                    for (j0, nb, ntiles) in region_chunks:
                        kve = gpool.tile([P, GCH, 2 * CDIM], dt.float16,
                                         tag="kve")
                        t0 = roff_tiles[j0]          # region tile offset
                        n_idx = ntiles * P
                        nc.gpsimd.dma_gather(
                            out_ap=kve[:, 0:ntiles, :], in_ap=table[:, :],
                            idxs_ap=idx_tile[:, t0 * 8:(t0 + ntiles) * 8],
                            num_idxs=n_idx, num_idxs_reg=n_idx,
                            elem_size=2 * CDIM, single_packet=False)
                        for j in range(j0, j0 + nb):
                            seg = roff_tiles[j + 1] - roff_tiles[j]
                            if seg == 0:
                                continue
                            dn = psD.tile([P, H + CDIM], dt.float32, tag="dn")
                            for s0 in range(0, seg, NT):
                                nt = min(NT, seg - s0)
                                kt0 = roff_tiles[j] - t0 + s0
                                g0 = (0 if is_lo else TOTLO // P) \
                                    + roff_tiles[j] + s0
                                sel, exwv = do_slab(kve, g0, kt0, j, nt)
                                for t in range(nt):
                                    nc.tensor.matmul(
                                        dn[:], sel[:, t, :], exwv[:, t, :],
                                        start=(s0 == 0 and t == 0),
                                        stop=(s0 + nt == seg and t == nt - 1))
                            if is_lo:
                                nc.scalar.copy(out=acc[:, j, :], in_=dn[:])
                            else:
                                nc.vector.tensor_tensor(
                                    out=acc[:, j, :], in0=acc[:, j, :],
                                    in1=dn[:], op=AluOp.add)
                                finalize(j)

                def finalize(j):
                    den = fpool.tile([P, H], dt.float32, tag="den")
                    nc.vector.tensor_scalar_max(out=den[:], in0=acc[:, j, 0:H],
                                                scalar1=1e-30)
                    rec = fpool.tile([P, H], dt.float32, tag="rec")
                    nc.vector.reciprocal(out=rec[:], in_=den[:])
                    s_sb = fpool.tile([P, CDIM], dt.float16, tag="s_sb")
                    nc.vector.tensor_tensor(
                        out=s_sb[:], in0=acc[:, j, H:H + CDIM],
                        in1=rec[:].unsqueeze(2).to_broadcast([P, H, DH]),
                        op=AluOp.mult)
                    st_ps = psF.tile([P, 2, P], dt.float16, tag="st_ps")
                    nc.tensor.transpose(st_ps[:, 0, :], s_sb[:, 0:P], ident[:])
                    nc.tensor.transpose(st_ps[:, 1, :], s_sb[:, P:2 * P], ident[:])
                    st_sb = fpool.tile([P, 2, P], dt.float16, tag="st_sb")
                    nc.scalar.copy(out=st_sb[:], in_=st_ps[:])
                    out_ps = psF.tile([P, CDIM], dt.float32, tag="out_ps")
                    nc.tensor.matmul(out_ps[:], st_sb[:, 0, :], wproj[:, 0, :],
                                     start=True, stop=False)
                    nc.tensor.matmul(out_ps[:], st_sb[:, 1, :], wproj[:, 1, :],
                                     start=False, stop=True)
                    fa_t = fpool.tile([P, CDIM], dt.float32, tag="fa_t")
                    nc.sync.dma_start(out=fa_t[:], in_=FaRes_t[j * P:(j + 1) * P, :])
                    res = fpool.tile([P, CDIM], dt.float32, tag="res")
                    nc.vector.tensor_tensor(out=res[:], in0=out_ps[:], in1=fa_t[:],
                                            op=AluOp.add)
                    nc.sync.dma_start(out=OUT_t[j * P:(j + 1) * P, :], in_=res[:])

                lo_tiles = np.concatenate([[0], np.cumsum(LO // P)]).astype(int)
                hi_tiles = np.concatenate([[0], np.cumsum(HI // P)]).astype(int)
                do_region(lo_chunks, kvloidx, KVlo, lo_tiles, True)
                do_region(hi_chunks, kvhiidx, KVhi, hi_tiles, False)

    nc.compile()
    return nc


TRACE = False          # set by test harness for NTFF profiling
LAST_RESULT = None     # BassKernelResults of the last run (for profiling)


def kernel(**inputs):
    global LAST_RESULT
    from concourse.bass_utils import run_bass_kernel_spmd

    meta, shared, per_core = preprocess(**inputs)
    nc = build_program(meta)
    in_maps = [dict(shared, **pc) for pc in per_core]
    res = run_bass_kernel_spmd(nc, in_maps, core_ids=list(range(NCORES)),
                               trace=TRACE)
    LAST_RESULT = res
    out = np.empty((NA, CDIM), F32)
    for m in range(NCORES):
        out[m * NAC:(m + 1) * NAC] = res.results[m]["OUT"][:NAC]
    return out
